# revision 5
# baseline (speedup 1.0000x reference)
"""NT-Xent / SimCLR contrastive loss on 8 Trainium2 NeuronCores.

Math (reference):
  z = concat(proj_1, proj_2)            # [2N, D], 2N=8192, D=128
  zn = z / ||z||                        # row L2-normalize
  sim = zn @ zn.T                       # [2N, 2N]
  denom_i   = sum_{j != i} exp(sim_ij / T)
  pos_i     = sim[i, (i+N) mod 2N]
  loss      = mean_i( log(denom_i) - pos_i / T )

Device decomposition: row-parallel over 8 cores; core c owns rows
[c*1024, (c+1)*1024). Each core receives the full z ROLLED by -1024*c
rows so its own rows are always samples 0..1023 (the gram stationary is
simply zn_t[:, :1024]); row sums and the positives pairing (distance
4096 mod 8192) are invariant under the roll.

Per core inputs: rolled z natural-packed [128, 64, 128] (z[128t+p, d]
at [p,t,d]) for row norms, rolled z^T [128(d), 8192(sample)] for the
GEMM. Normalization is pipelined in 5 chunks (1024,1024,2048,2048,2048
cols): DVE square -> DVE reduce -> ACT sqrt -> DVE reciprocal+cast ->
padded DMA-transpose (on the otherwise-idle Activation HWDGE queue) ->
DRAM bounce -> DMA broadcast (gpsimd queue) -> DVE multiply. Bulk
loads go chunk-interleaved on the sync HWDGE queue. ACT runs exactly
three table loads (sqrt preloaded during the input DMA, exp, ln).

Gram phase: per 2048-col PSUM chunk (x2 double-buffered): 4 matmuls
(K=128, N=512, bf16) + one fused ScalarE exp/row-sum
activation(Exp, scale=1/T, accum_out). denom = rowsum - e^2 (self),
log on ScalarE. positives: DVE <Zn1,Zn2> under the gram phase.
Host: loss = (sum log(denom) - (2/T)*<Zn1,Zn2>) / 8192.
"""

import numpy as np

P = 128          # partitions / feature dim
NS = 8192        # total samples (2N)
D = 128          # feature dim
NCORES = 8
RB = NS // NCORES    # 1024 rows per core
MT = RB // P         # 8 m-tiles per core
NT = NS // P         # 64 sample tiles
TEMP = 0.5
INV_T = 1.0 / TEMP   # 2.0
NFREE = 512          # matmul moving free dim
ACT_CHUNK = 2048     # ScalarE exp chunk (4 PSUM banks)
NQ = NS // ACT_CHUNK       # 4 PSUM column chunks
NORM_BOUNDS = [0, 1024, 2048, 4096, 6144, 8192]  # norm pipeline chunks

_CACHE = {}


def _ensure_paths():
    import sys
    for p in ("/root/.axon_site", "/root/.axon_site/_ro/trn_rl_repo",
              "/root/.axon_site/_ro/pypackages", "/opt/trn_rl_repo", "/opt/pypackages"):
        if p not in sys.path:
            sys.path.append(p)


def _build():
    _ensure_paths()
    import concourse.bass as bass
    import concourse.bacc as bacc
    import concourse.mybir as mybir
    import concourse.tile as tile

    dt_bf = mybir.dt.bfloat16
    dt_f32 = mybir.dt.float32
    AFT = mybir.ActivationFunctionType
    AX = mybir.AxisListType

    nc = bacc.Bacc("TRN2", target_bir_lowering=False, debug=False,
                   num_devices=NCORES)

    znat_d = nc.dram_tensor("znat", [P, NT, P], dt_bf, kind="ExternalInput")
    zt_d = nc.dram_tensor("zt", [P, NS], dt_bf, kind="ExternalInput")
    ld_d = nc.dram_tensor("out_ld", [P, MT], dt_f32, kind="ExternalOutput")
    pos_d = nc.dram_tensor("out_pos", [P, 1], dt_f32, kind="ExternalOutput")
    ninv_dram = nc.dram_tensor("ninv_row_scratch", [1, NS], dt_bf)

    with tile.TileContext(nc) as tc:
        with (
            tc.tile_pool(name="big", bufs=1) as big,
            tc.tile_pool(name="work", bufs=2) as work,
            tc.tile_pool(name="psum", bufs=2, space=bass.MemorySpace.PSUM) as psum,
        ):
            zt = big.tile([P, NS], dt_bf, tag="zt")        # z^T raw
            zn_t = big.tile([P, NS], dt_bf, tag="zn_t")    # zn^T
            zna = big.tile([P, NT, P], dt_bf, tag="zna")   # z natural packed
            ninv_b = big.tile([P, NS], dt_bf, tag="ninv_b")
            ss = big.tile([P, NT], dt_f32, tag="ss")       # sum of squares
            sn = big.tile([P, NT], dt_f32, tag="sn")       # sqrt(ss)
            rinv = big.tile([P, NT], dt_f32, tag="rinv")   # 1/sqrt(ss)
            rs = big.tile([P, MT * NQ], dt_f32, tag="rs")  # exp row-sum accums

            # ---- sqrt table preload; runs while the input DMAs stream ----
            scr = work.tile([P, 1], dt_f32, tag="scr")
            nc.gpsimd.memset(scr[:], 1.0)
            scr2 = work.tile([P, 1], dt_f32, tag="scr2")
            nc.scalar.activation(scr2[:], scr[:], AFT.Sqrt)

            # ---- bulk input DMAs, chunk-interleaved, sync HWDGE queue ----
            for h in range(len(NORM_BOUNDS) - 1):
                c0, c1 = NORM_BOUNDS[h], NORM_BOUNDS[h + 1]
                nc.sync.dma_start(zna[:, c0 // P:c1 // P, :],
                                  znat_d[:, c0 // P:c1 // P, :])
                nc.sync.dma_start(zt[:, c0:c1], zt_d[:, c0:c1])

            def norm_chunk(h):
                c0, c1 = NORM_BOUNDS[h], NORM_BOUNDS[h + 1]
                ct = (c1 - c0) // P                        # tiles in chunk
                tsl = slice(c0 // P, c1 // P)
                csl = slice(c0, c1)
                sq = work.tile([P, ct, P], dt_bf, tag=f"sq{ct}")
                nc.vector.tensor_mul(sq[:], zna[:, tsl, :], zna[:, tsl, :])
                nc.vector.reduce_sum(ss[:, tsl], sq[:], axis=AX.X)
                nc.scalar.activation(sn[:, tsl], ss[:, tsl], AFT.Sqrt)
                nc.vector.reciprocal(rinv[:, tsl], sn[:, tsl])
                nvb = work.tile([P, P], dt_bf, tag="nvb")  # cols ct..P: pad
                nc.gpsimd.memset(nvb[:, ct:], 1.0)
                nc.vector.tensor_copy(nvb[:, :ct], rinv[:, tsl])
                nvt = work.tile([P, P], dt_bf, tag="nvt")
                # Activation HWDGE queue: empty, so transposes never queue
                # behind the bulk loads (costs no ACT engine cycles)
                nc.scalar.dma_start_transpose(nvt[:], nvb[:])
                nc.gpsimd.dma_start(ninv_dram[0:1, csl], nvt[0:ct, :])
                nc.gpsimd.dma_start(ninv_b[:, csl],
                                    ninv_dram[0:1, csl].broadcast_to(
                                        [P, c1 - c0]))
                nc.vector.tensor_mul(zn_t[:, csl], zt[:, csl],
                                     ninv_b[:, csl])

            for h in range(len(NORM_BOUNDS) - 1):
                norm_chunk(h)

            # ---- positives: <Zn1, Zn2> (overlaps the gram phase) ----
            pp = big.tile([P, NS // 2], dt_bf, tag="pp")
            posb = big.tile([P, 1], dt_f32, tag="posb")
            nc.vector.tensor_mul(pp[:], zn_t[:, :NS // 2], zn_t[:, NS // 2:])
            nc.vector.reduce_sum(posb[:], pp[:], axis=AX.X)
            nc.gpsimd.dma_start(pos_d[:], posb[:])

            # ---- gram rows + fused exp/row-sum ----
            nmm = ACT_CHUNK // NFREE               # 4 matmuls per chunk
            for q in range(NQ):
                for m in range(MT):
                    ps = psum.tile([P, ACT_CHUNK], dt_f32, tag="ps")
                    for j in range(nmm):
                        b0 = q * ACT_CHUNK + j * NFREE
                        nc.tensor.matmul(ps[:, j * NFREE:(j + 1) * NFREE],
                                         zn_t[:, m * P:(m + 1) * P],
                                         zn_t[:, b0:b0 + NFREE],
                                         start=True, stop=True)
                    esc = work.tile([P, ACT_CHUNK], dt_bf, tag="esc")
                    idx = q * MT + m
                    nc.scalar.activation(esc[:], ps[:], AFT.Exp, scale=INV_T,
                                         accum_out=rs[:, idx:idx + 1])

            # ---- denominators -> log ----
            rsum = big.tile([P, MT], dt_f32, tag="rsum")
            nc.vector.reduce_sum(rsum[:], rs[:].rearrange("p (q m) -> p m q",
                                                          m=MT), axis=AX.X)
            den = big.tile([P, MT], dt_f32, tag="den")
            nc.vector.tensor_scalar_add(den[:], rsum[:],
                                        -float(np.exp(2.0)))
            ldb = big.tile([P, MT], dt_f32, tag="ldb")
            nc.scalar.activation(ldb[:], den[:], AFT.Ln)
            nc.gpsimd.dma_start(ld_d[:], ldb[:])

    nc.compile()
    return nc


def get_nc():
    if "nc" not in _CACHE:
        _CACHE["nc"] = _build()
    return _CACHE["nc"]


def make_in_maps(proj_1: np.ndarray, proj_2: np.ndarray):
    import ml_dtypes
    z = np.concatenate([np.asarray(proj_1), np.asarray(proj_2)], axis=0)
    zb = z.astype(ml_dtypes.bfloat16)
    in_maps = []
    for c in range(NCORES):
        zc = np.roll(zb, -RB * c, axis=0)
        znat = np.ascontiguousarray(zc.reshape(NT, P, P).transpose(1, 0, 2))
        ztr = np.ascontiguousarray(zc.T)
        in_maps.append({"znat": znat, "zt": ztr})
    return in_maps


def finish(results) -> np.ndarray:
    ld_sum = 0.0
    pos_vals = []
    for r in results:
        ld_sum += float(np.asarray(r["out_ld"], dtype=np.float64).sum())
        pos_vals.append(float(np.asarray(r["out_pos"], dtype=np.float64).sum()))
    pos_dot = float(np.mean(pos_vals))
    loss = (ld_sum - 2.0 * INV_T * pos_dot) / float(NS)
    return np.float32(loss)


def kernel(proj_1: np.ndarray, proj_2: np.ndarray) -> np.ndarray:
    _ensure_paths()
    from concourse.bass_utils import run_bass_kernel_spmd
    nc = get_nc()
    in_maps = make_in_maps(proj_1, proj_2)
    res = run_bass_kernel_spmd(nc, in_maps, core_ids=list(range(NCORES)))
    return finish(res.results)
